# revision 19
# baseline (speedup 1.0000x reference)
"""Trainium2 Bass kernel for a meshed-memory-style transformer decoder block.

Data-parallel over batch (B=8) across 8 NeuronCores, one batch element per
core.  Activations live in "transposed" layout [feature, token] = chunks of
[128, 512] so every matmul's contraction dim sits on SBUF partitions and
weights load directly as stationary operands.  Matmuls in bf16 (fp32 PSUM);
LayerNorm stats / softmax / gating in fp32.

LayerNorm in transposed layout: per-token sums via ones-vector matmuls; the
per-token mean/rstd rows are broadcast across partitions with a DRAM-roundtrip
broadcast DMA.  LN gain/bias folded into downstream weights/biases on host.

Attention: scores computed transposed S^T[k, q]; softmax without
max-subtraction (scores are O(1)); masked entries underflow to exactly 0.
Normalizer Z from a ones-column appended to the value stationary; division
applied after the p@v matmul.
"""

import os

import numpy as np
import ml_dtypes

B, S, E, H, D, F = 8, 512, 768, 12, 64, 3072
P = 128
NE = E // P   # 6
NS = S // P   # 4
NF = F // P   # 24
EPS = 1e-5
BF16 = ml_dtypes.bfloat16

LAST_RESULTS = None
LAST_NC = None


def _bcast_ap(bass_mod, row_ap, parts):
    """Partition-broadcast a [1, ...] DRAM AP to [parts, ...]."""
    return bass_mod.AP(
        tensor=row_ap.tensor,
        offset=row_ap.offset,
        ap=[[0, parts]] + list(row_ap.ap)[1:],
    )


def _build(tau_f: float, causal: bool = False):
    import concourse.bass as bass
    import concourse.tile as tile
    from concourse import mybir, bacc
    from concourse.masks import make_identity

    nc = bacc.Bacc("TRN2", target_bir_lowering=False, debug=False, num_devices=B)
    f32 = mybir.dt.float32
    bf16 = mybir.dt.bfloat16

    dram = {}
    for name, shape, dt_ in [
        ("x", [S, E], f32), ("enc", [3, S, E], f32), ("m8T", [S, S], bf16),
        ("menc", [S, 1], f32), ("mq2", [2, S], f32), ("dbc", [3, E], bf16),
        ("biasE", [E, 10], f32), ("biasF", [P, NF], f32),
        ("wq", [E, E], bf16), ("wk", [E, E], bf16), ("wv", [E, E], bf16),
        ("wo", [E, E], bf16), ("fcq", [E, E], bf16), ("fck", [E, E], bf16),
        ("fcv", [E, E], bf16), ("cpj", [E, E], bf16),
        ("gw", [3, NE, P, 2 * E], bf16), ("cfc", [NF, P, E], bf16),
        ("cp2", [NE, P, F], bf16),
    ]:
        dram[name] = nc.dram_tensor(name, shape, dt_, kind="ExternalInput")
    dram["out"] = nc.dram_tensor("out", [S, E], f32, kind="ExternalOutput")
    dram["present"] = nc.dram_tensor("present", [2, H, S, D], f32,
                                     kind="ExternalOutput")

    with tile.TileContext(nc) as tc:
        import contextlib
        with contextlib.ExitStack() as ctx:
            sb = ctx.enter_context(tc.tile_pool(name="sb", bufs=2))
            ps = ctx.enter_context(tc.tile_pool(name="ps", bufs=2, space="PSUM"))
            dr = ctx.enter_context(tc.tile_pool(name="dr", bufs=2, space="DRAM"))
            _emit(nc, sb, ps, dr, bass, mybir, make_identity, tau_f,
                  causal, dram)

    nc.compile()
    return nc


def _emit(nc, sb, ps, dr, bass, mybir, make_identity, tau_f,
                  causal, dram):
    AF = mybir.ActivationFunctionType
    ALU = mybir.AluOpType
    f32 = mybir.dt.float32
    bf16 = mybir.dt.bfloat16

    # bias column indices in biasE
    (BI_DQ, BI_DK, BI_BO, BI_DCQ, BI_DCK, BI_CPJ, BI_A1, BI_A2, BI_A3,
     BI_C2) = range(10)
    gate_bias = [BI_A1, BI_A2, BI_A3]

    def bcast(row_ap, parts=P):
        return _bcast_ap(bass, row_ap, parts)

    # ---------------- constants ----------------
    ident = sb.tile([P, P], f32, name="ident", tag="ident", bufs=1)
    make_identity(nc, ident)
    ones_col = sb.tile([P, 1], bf16, name="ones_col", tag="ones_col", bufs=1)
    nc.vector.memset(ones_col, 1.0)
    eps_t = sb.tile([1, 1], f32, name="eps_t", tag="eps_t", bufs=1)
    nc.vector.memset(eps_t, EPS)

    bias_t = []
    for c in range(NE):
        t = sb.tile([P, 10], f32, name=f"biasE{c}", tag=f"biasE{c}", bufs=1)
        nc.sync.dma_start(out=t, in_=dram["biasE"].ap()[c * P:(c + 1) * P, :])
        bias_t.append(t)
    biasF_t = sb.tile([P, NF], f32, name="biasF", tag="biasF", bufs=1)
    nc.sync.dma_start(out=biasF_t, in_=dram["biasF"].ap())

    menc_t = []
    for kc in range(NS):
        t = sb.tile([P, 1], f32, name=f"menc{kc}", tag=f"menc{kc}", bufs=1)
        nc.sync.dma_start(out=t, in_=dram["menc"].ap()[kc * P:(kc + 1) * P, :])
        menc_t.append(t)
    m8T_t = []
    for kc in range(NS):
        t = sb.tile([P, S], bf16, name=f"m8T{kc}", tag=f"m8T{kc}", bufs=1)
        nc.sync.dma_start(out=t, in_=dram["m8T"].ap()[kc * P:(kc + 1) * P, :])
        m8T_t.append(t)

    # ---------------- helpers ----------------
    def load_wrows(name, w_ap):
        rows = []
        for ec in range(NE):
            t = sb.tile([P, E], bf16, name=f"{name}_w{ec}", tag="wrow", bufs=6)
            nc.sync.dma_start(out=t, in_=w_ap[ec * P:(ec + 1) * P, :])
            rows.append(t)
        return rows

    def wwide(name, w_tiled_ap, oc, nchunk, tag, bufs):
        """w_tiled_ap: [n_oc, 128, nchunk*128] host-pretiled (p-major).
        One DMA loads the whole contraction for out-chunk oc; slice [:, ec, :]
        is the [128,128] stationary for contraction chunk ec."""
        t = sb.tile([P, nchunk, P], bf16, name=f"{name}_ww{oc}", tag=tag,
                    bufs=bufs)
        nc.sync.dma_start(out=t, in_=w_tiled_ap[oc].rearrange(
            "p (a c) -> p a c", a=nchunk))
        return t

    def transpose_grid(name, srcs, n_out, dst_dtype, dst_tag, dst_bufs,
                       src_outer=True):
        """dst-outer; per dst, srcs are transposed in groups of 4 into one
        PSUM tile and evicted with a single wide copy.  ALL srcs must have
        enough bufs to stay live for the whole grid."""
        del src_outer
        dsts = []
        for j in range(n_out):
            dst = sb.tile([P, len(srcs) * P], dst_dtype, name=f"{name}_t{j}",
                          tag=dst_tag, bufs=dst_bufs)
            for g0 in range(0, len(srcs), 4):
                gs = srcs[g0:g0 + 4]
                ptag = "mm" if len(gs) > 2 else "ptm"
                pt = ps.tile([P, len(gs) * P], f32, name=f"{name}_pt{j}_{g0}",
                             tag=ptag, bufs=3 if ptag == "mm" else 2)
                for i, st in enumerate(gs):
                    nc.tensor.transpose(pt[:, i * P:(i + 1) * P],
                                        st[:, j * P:(j + 1) * P], ident)
                nc.scalar.activation(
                    dst[:, g0 * P:(g0 + len(gs)) * P], pt, AF.Copy)
            dsts.append(dst)
        return dsts

    def layer_norm(name, src, src_is_f32, keep_bf_tag=None):
        """src: NE tiles [P, S] (f32 or bf16). Returns (xn bf16, bf16 casts)."""
        if src_is_f32:
            bf = []
            for c in range(NE):
                tg = keep_bf_tag or "lnbf"
                t = sb.tile([P, S], bf16, name=f"{name}_bf{c}", tag=tg,
                            bufs=NE if keep_bf_tag else 3)
                nc.scalar.activation(t, src[c], AF.Copy)
                bf.append(t)
        else:
            bf = src
        sq = []
        for c in range(NE):
            t = sb.tile([P, S], bf16, name=f"{name}_sq{c}", tag="lnsq", bufs=2)
            nc.scalar.activation(t, bf[c], AF.Square)
            sq.append(t)
        s1 = ps.tile([1, S], f32, name=f"{name}_s1", tag="psmall", bufs=2)
        s2 = ps.tile([1, S], f32, name=f"{name}_s2", tag="psmall", bufs=2)
        for c in range(NE):
            nc.tensor.matmul(s1, ones_col, bf[c], start=(c == 0),
                             stop=(c == NE - 1))
        for c in range(NE):
            nc.tensor.matmul(s2, ones_col, sq[c], start=(c == 0),
                             stop=(c == NE - 1))
        mu = sb.tile([1, S], f32, name=f"{name}_mu", tag="strow", bufs=3)
        nc.scalar.activation(mu, s1, AF.Copy, scale=1.0 / E)
        e2 = sb.tile([1, S], f32, name=f"{name}_e2", tag="strow", bufs=3)
        nc.scalar.activation(e2, s2, AF.Copy, scale=1.0 / E)
        musq = sb.tile([1, S], f32, name=f"{name}_musq", tag="lntmp", bufs=2)
        nc.vector.tensor_mul(musq, mu, mu)
        nc.vector.tensor_sub(e2, e2, musq)          # e2 <- var
        nc.vector.tensor_scalar_max(e2, e2, 0.0)
        sd = sb.tile([1, S], f32, name=f"{name}_sd", tag="lntmp", bufs=2)
        nc.scalar.activation(sd, e2, AF.Sqrt, bias=eps_t)
        r = sb.tile([1, S], f32, name=f"{name}_r", tag="strow", bufs=3)
        nc.vector.reciprocal(r, sd)
        drow = dr.tile([2, S], f32, name=f"{name}_drow", tag="drow", bufs=4)
        nc.sync.dma_start(out=drow[0:1, :], in_=mu)
        nc.sync.dma_start(out=drow[1:2, :], in_=r)
        mubc = sb.tile([P, S], f32, name=f"{name}_mubc", tag="bc32", bufs=2)
        nc.sync.dma_start(out=mubc, in_=bcast(drow[0:1, :]))
        rbc = sb.tile([P, S], f32, name=f"{name}_rbc", tag="bc32", bufs=2)
        nc.sync.dma_start(out=rbc, in_=bcast(drow[1:2, :]))
        xn = []
        for c in range(NE):
            tmp = sb.tile([P, S], f32, name=f"{name}_tmp{c}", tag="gtA",
                          bufs=2)
            nc.vector.tensor_sub(tmp, src[c], mubc)
            t = sb.tile([P, S], bf16, name=f"{name}_n{c}", tag="ln_n", bufs=NE)
            nc.vector.tensor_mul(t, tmp, rbc)
            xn.append(t)
        return xn, bf

    def proj_fm(name, wrows, rhs, bias_idx, func, out_dtype, out_tag,
                out_bufs, evict="act"):
        outs = []
        nin = len(wrows)
        for oc in range(NE):
            pt = ps.tile([P, S], f32, name=f"{name}_ps{oc}", tag="mm", bufs=3)
            for ec in range(nin):
                nc.tensor.matmul(pt, wrows[ec][:, oc * P:(oc + 1) * P],
                                 rhs[ec], start=(ec == 0), stop=(ec == nin - 1))
            o = sb.tile([P, S], out_dtype, name=f"{name}_o{oc}", tag=out_tag,
                        bufs=out_bufs)
            if evict == "dve":
                nc.vector.tensor_scalar(o, pt,
                                        bias_t[oc][:, bias_idx:bias_idx + 1],
                                        None, op0=ALU.add)
            else:
                f = AF.Identity if func == AF.Copy else func
                nc.scalar.activation(o, pt, f,
                                     bias=bias_t[oc][:, bias_idx:bias_idx + 1])
            outs.append(o)
        return outs

    def proj_tm(name, wrows, xn, dbc, want_f32, want_va):
        f32s, vas = [], []
        for sc in range(NS):
            pa = ps.tile([P, 512], f32, name=f"{name}_pa{sc}", tag="mm", bufs=3)
            pb = ps.tile([P, 256], f32, name=f"{name}_pb{sc}", tag="ptm",
                         bufs=2)
            for ec in range(NE):
                lhsT = xn[ec][:, sc * P:(sc + 1) * P]
                nc.tensor.matmul(pa, lhsT, wrows[ec][:, 0:512],
                                 start=(ec == 0), stop=(ec == NE - 1))
                nc.tensor.matmul(pb, lhsT, wrows[ec][:, 512:768],
                                 start=(ec == 0), stop=(ec == NE - 1))
            ft, va = None, None
            if want_f32:
                ft = sb.tile([P, E], f32, name=f"{name}_f{sc}", tag="ptok",
                             bufs=2)
                nc.vector.tensor_add(ft[:, 0:512], pa, dbc[:, 0:512])
                nc.vector.tensor_add(ft[:, 512:768], pb, dbc[:, 512:768])
                f32s.append(ft)
            if want_va:
                va = sb.tile([P, H, D + 1], bf16, name=f"{name}_va{sc}",
                             tag="va", bufs=NS)
                nc.gpsimd.memset(va, 1.0)
                if want_f32:
                    nc.scalar.activation(
                        va[:, :, 0:D], ft.rearrange("p (h d) -> p h d", h=H),
                        AF.Copy)
                else:
                    nc.vector.tensor_add(
                        va[:, 0:8, 0:D],
                        pa.rearrange("p (h d) -> p h d", h=8),
                        dbc[:, 0:512].rearrange("p (h d) -> p h d", h=8))
                    nc.vector.tensor_add(
                        va[:, 8:12, 0:D],
                        pb.rearrange("p (h d) -> p h d", h=4),
                        dbc[:, 512:768].rearrange("p (h d) -> p h d", h=4))
                vas.append(va)
        return f32s, vas

    def pres_out_ap(which, sc):
        base = dram["present"].ap()
        off = which * H * S * D + sc * P * D
        return bass.AP(tensor=base.tensor, offset=off,
                       ap=[[D, P], [S * D, H], [1, D]])

    def attention(name, qT, kT, va_list, is_self, causal=False):
        oT = [sb.tile([P, S], bf16, name=f"{name}_oT{c}", tag="bfT", bufs=12)
              for c in range(NE)]
        dz = dr.tile([H, S], f32, name=f"{name}_dz", tag="drz", bufs=2)
        for h in range(H):
            ch, ro = h // 2, (h % 2) * 64
            ets = []
            for kc in range(NS):
                q0 = kc * P if (is_self and causal) else 0
                pt = ps.tile([P, S], f32, name=f"{name}_s{h}_{kc}", tag="mm",
                             bufs=3)
                nc.tensor.matmul(pt[:, q0:],
                                 kT[ch][ro:ro + 64, kc * P:(kc + 1) * P],
                                 qT[ch][ro:ro + 64, q0:], start=True,
                                 stop=True)
                et = sb.tile([P, S], bf16, name=f"{name}_e{h}_{kc}", tag="ET",
                             bufs=4)
                if is_self and causal:
                    # only the diagonal block needs masking; q < q0 is never
                    # read downstream
                    nc.vector.tensor_add(pt[:, q0:q0 + P], pt[:, q0:q0 + P],
                                         m8T_t[kc][:, q0:q0 + P])
                    nc.scalar.activation(et[:, q0:], pt[:, q0:], AF.Exp,
                                         scale=0.125)
                elif is_self:
                    nc.vector.tensor_add(pt, pt, m8T_t[kc])
                    nc.scalar.activation(et, pt, AF.Exp, scale=0.125)
                else:
                    nc.scalar.activation(et, pt, AF.Exp, scale=0.125,
                                         bias=menc_t[kc])
                ets.append(et)
            po = ps.tile([D + 1, S], f32, name=f"{name}_po{h}", tag="psmall",
                         bufs=2)
            for kc in range(NS):
                q0 = kc * P if (is_self and causal) else 0
                nc.tensor.matmul(po[:, q0:], va_list[kc][:, h, :],
                                 ets[kc][:, q0:], start=(kc == 0),
                                 stop=(kc == NS - 1))
            if h % 2 == 0:
                nc.scalar.activation(oT[ch][ro:ro + 64, :], po[0:D, :],
                                     AF.Copy)
            else:
                nc.vector.tensor_copy(oT[ch][ro:ro + 64, :], po[0:D, :])
            zt = sb.tile([1, S], f32, name=f"{name}_z{h}", tag="zrow", bufs=2)
            nc.vector.reciprocal(zt, po[D:D + 1, :])
            nc.sync.dma_start(out=dz[h:h + 1, :], in_=zt)
        onT = []
        for c in range(NE):
            rzbc = sb.tile([P, S], f32, name=f"{name}_rzbc{c}", tag="bc32",
                           bufs=2)
            for i in range(2):
                nc.sync.dma_start(out=rzbc[64 * i:64 * (i + 1), :],
                                  in_=bcast(dz[2 * c + i:2 * c + i + 1, :], 64))
            t = sb.tile([P, S], bf16, name=f"{name}_on{c}", tag="bfT", bufs=12)
            nc.vector.tensor_mul(t, oT[c], rzbc)
            onT.append(t)
        return onT

    # ---------------- phase 1: x -> x^T, LN1(x) ----------------
    xa = []
    for sc in range(NS):
        t = sb.tile([P, E], f32, name=f"xa{sc}", tag="tload", bufs=4)
        nc.sync.dma_start(out=t, in_=dram["x"].ap()[sc * P:(sc + 1) * P, :])
        xa.append(t)
    xT = transpose_grid("xT", xa, NE, f32, "xT", NE)
    xn, _ = layer_norm("lnx", xT, True)

    # ---------------- phase 2: self q/k/v (+present) ----------------
    dbck = sb.tile([P, E], bf16, name="dbck", tag="dbc", bufs=2)
    nc.sync.dma_start(out=dbck, in_=bcast(dram["dbc"].ap()[0:1, :]))
    dbcv = sb.tile([P, E], bf16, name="dbcv", tag="dbc", bufs=2)
    nc.sync.dma_start(out=dbcv, in_=bcast(dram["dbc"].ap()[1:2, :]))

    wq_rows = load_wrows("wq", dram["wq"].ap())
    qT = proj_fm("q", wq_rows, xn, BI_DQ, AF.Copy, bf16, "qTt", NE,
                 evict="dve")
    wk_rows = load_wrows("wk", dram["wk"].ap())
    kT = proj_fm("k", wk_rows, xn, BI_DK, AF.Copy, bf16, "kTt", NE,
                 evict="dve")
    ktokf, _ = proj_tm("ktok", wk_rows, xn, dbck, True, False)
    for sc in range(NS):
        nc.sync.dma_start(out=pres_out_ap(0, sc),
                          in_=ktokf[sc].rearrange("p (h d) -> p h d", h=H))
    wv_rows = load_wrows("wv", dram["wv"].ap())
    vtokf, va_self = proj_tm("vtok", wv_rows, xn, dbcv, True, True)
    for sc in range(NS):
        nc.sync.dma_start(out=pres_out_ap(1, sc),
                          in_=vtokf[sc].rearrange("p (h d) -> p h d", h=H))

    # ---------------- phase 3: self attention + out proj + residual --------
    onT = attention("sa", qT, kT, va_self, True, causal=causal)
    wo_rows = load_wrows("wo", dram["wo"].ap())
    for oc in range(NE):
        pt = ps.tile([P, S], f32, name=f"op_ps{oc}", tag="mm", bufs=3)
        for ec in range(NE):
            nc.tensor.matmul(pt, wo_rows[ec][:, oc * P:(oc + 1) * P], onT[ec],
                             start=(ec == 0), stop=(ec == NE - 1))
        # a = x + sa, in place into xT
        nc.vector.scalar_tensor_tensor(
            xT[oc], pt, bias_t[oc][:, BI_BO:BI_BO + 1], xT[oc],
            op0=ALU.add, op1=ALU.add)
    aT = xT

    # ---------------- phase 4: LN1(a), cross-q ----------------
    anT, abf = layer_norm("lna", aT, True, keep_bf_tag="abf")
    fcq_rows = load_wrows("fcq", dram["fcq"].ap())
    qcT = proj_fm("qc", fcq_rows, anT, BI_DCQ, AF.Copy, bf16, "qTt", NE)

    # ---------------- phase 5: cross streams + incremental gates ----------
    dbccv = sb.tile([P, E], bf16, name="dbccv", tag="dbc", bufs=2)
    nc.sync.dma_start(out=dbccv, in_=bcast(dram["dbc"].ap()[2:3, :]))
    acc = []
    for j in range(3):
        ea = []
        for sc in range(NS):
            t = sb.tile([P, E], f32, name=f"e{j}a{sc}", tag="tload", bufs=4)
            nc.sync.dma_start(out=t,
                              in_=dram["enc"].ap()[j, sc * P:(sc + 1) * P, :])
            ea.append(t)
        encT = transpose_grid(f"encT{j}", ea, NE, bf16, "bfT", 12)
        enT, _ = layer_norm(f"lne{j}", encT, False)
        fck_rows = load_wrows(f"fck{j}", dram["fck"].ap())
        kjT = proj_fm(f"kc{j}", fck_rows, enT, BI_DCK, AF.Copy, bf16, "kTt",
                      NE, evict="dve")
        fcv_rows = load_wrows(f"fcv{j}", dram["fcv"].ap())
        _, va_j = proj_tm(f"vc{j}", fcv_rows, enT, dbccv, False, True)
        onjT = attention(f"ca{j}", qcT, kjT, va_j, False)
        cpj_rows = load_wrows(f"cpj{j}", dram["cpj"].ap())
        ejT = proj_fm(f"ej{j}", cpj_rows, onjT, BI_CPJ, AF.Copy, bf16, "bfT",
                      12, evict="dve")

        # gate j: alpha = sigmoid([a, e_j] @ gw_j + ab_j); accumulate
        rhs12 = abf + ejT
        for oc in range(NE):
            pt = ps.tile([P, S], f32, name=f"al{j}_ps{oc}", tag="mm", bufs=3)
            gww = wwide(f"gw{j}", dram["gw"].ap()[j], oc, 2 * NE, "wgw", 2)
            for ec in range(2 * NE):
                nc.tensor.matmul(pt, gww[:, ec, :], rhs12[ec], start=(ec == 0),
                                 stop=(ec == 2 * NE - 1))
            al = sb.tile([P, S], f32, name=f"al{j}_{oc}", tag="alpha", bufs=2)
            nc.scalar.activation(
                al, pt, AF.Sigmoid,
                bias=bias_t[oc][:, gate_bias[j]:gate_bias[j] + 1])
            if tau_f == 0.0:
                # gate = e + alpha * (a - e)
                d_ = sb.tile([P, S], f32, name=f"g{j}d{oc}", tag="gtA", bufs=2)
                nc.vector.tensor_sub(d_, aT[oc], ejT[oc])
                m_ = sb.tile([P, S], f32, name=f"g{j}m{oc}", tag="gtB", bufs=2)
                nc.vector.tensor_mul(m_, al, d_)
                if j == 0:
                    at = sb.tile([P, S], f32, name=f"acc{oc}", tag="acc",
                                 bufs=NE)
                    nc.vector.tensor_add(at, m_, ejT[oc])
                    acc.append(at)
                else:
                    g_ = sb.tile([P, S], f32, name=f"g{j}g{oc}", tag="gtA",
                                 bufs=2)
                    nc.vector.tensor_add(g_, m_, ejT[oc])
                    nc.vector.tensor_add(acc[oc], acc[oc], g_)
            else:
                # gate = alpha*lm*a + (1-alpha)*vm*e
                lm = sb.tile([P, S], f32, name=f"g{j}lm{oc}", tag="gtA",
                             bufs=2)
                nc.vector.tensor_scalar(lm, al, tau_f, None, op0=ALU.is_gt)
                t1 = sb.tile([P, S], f32, name=f"g{j}t1{oc}", tag="gtB",
                             bufs=2)
                nc.vector.tensor_mul(t1, al, lm)
                nc.vector.tensor_mul(t1, t1, aT[oc])
                bm = sb.tile([P, S], f32, name=f"g{j}bm{oc}", tag="gtA",
                             bufs=2)
                nc.vector.tensor_scalar(bm, al, -1.0, 1.0, op0=ALU.mult,
                                        op1=ALU.add)
                vm = sb.tile([P, S], f32, name=f"g{j}vm{oc}", tag="alpha",
                             bufs=2)
                nc.vector.tensor_scalar(vm, al, 1.0 - tau_f, None,
                                        op0=ALU.is_lt)
                nc.vector.tensor_mul(bm, bm, vm)
                nc.vector.tensor_mul(bm, bm, ejT[oc])
                if j == 0:
                    at = sb.tile([P, S], f32, name=f"acc{oc}", tag="acc",
                                 bufs=NE)
                    nc.vector.tensor_add(at, t1, bm)
                    acc.append(at)
                else:
                    nc.vector.tensor_add(t1, t1, bm)
                    nc.vector.tensor_add(acc[oc], acc[oc], t1)

    # aq = (g1+g2+g3)/sqrt(3) * mask_queries  (in place into acc)
    mqs_bc = sb.tile([P, S], f32, name="mqs_bc", tag="bc32", bufs=2)
    nc.sync.dma_start(out=mqs_bc, in_=bcast(dram["mq2"].ap()[0:1, :]))
    for oc in range(NE):
        nc.vector.tensor_mul(acc[oc], acc[oc], mqs_bc)
    aqT = acc

    # ---------------- phase 6: MLP ----------------
    aqn, _ = layer_norm("lnq", aqT, True)
    z = []
    for oc in range(NF):
        pt = ps.tile([P, S], f32, name=f"cfc_ps{oc}", tag="mm", bufs=3)
        cfcw = wwide("cfc", dram["cfc"].ap(), oc, NE, "wrow", 6)
        for ec in range(NE):
            nc.tensor.matmul(pt, cfcw[:, ec, :], aqn[ec], start=(ec == 0),
                             stop=(ec == NE - 1))
        ht = sb.tile([P, S], bf16, name=f"ht{oc}", tag="ht", bufs=2)
        nc.scalar.activation(ht, pt, AF.Identity, bias=biasF_t[:, oc:oc + 1])
        u = sb.tile([P, S], bf16, name=f"gu{oc}", tag="gt2", bufs=2)
        nc.vector.tensor_mul(u, ht, ht)
        nc.vector.tensor_mul(u, ht, u)                      # u <- h^3
        nc.vector.scalar_tensor_tensor(u, u, 0.044715, ht, op0=ALU.mult,
                                       op1=ALU.add)         # u <- inner
        th = sb.tile([P, S], bf16, name=f"gth{oc}", tag="gth", bufs=2)
        nc.scalar.activation(th, u, AF.Tanh, scale=0.7978845608028654)
        zt = sb.tile([P, S], bf16, name=f"z{oc}", tag="z", bufs=NF)
        nc.vector.scalar_tensor_tensor(zt, th, 1.0, ht, op0=ALU.add,
                                       op1=ALU.mult)        # 2*gelu(h)
        z.append(zt)

    mq_bc = sb.tile([P, S], f32, name="mq_bc", tag="bc32", bufs=2)
    nc.sync.dma_start(out=mq_bc, in_=bcast(dram["mq2"].ap()[1:2, :]))
    outT = []
    for oc in range(NE):
        pt = ps.tile([P, S], f32, name=f"cp2_ps{oc}", tag="mm", bufs=3)
        cp2w = wwide("cp2", dram["cp2"].ap(), oc, NF, "wcp2", 2)
        for ec in range(NF):
            nc.tensor.matmul(pt, cp2w[:, ec, :], z[ec], start=(ec == 0),
                             stop=(ec == NF - 1))
        t2 = sb.tile([P, S], f32, name=f"outT{oc}", tag="outT", bufs=NE)
        nc.vector.scalar_tensor_tensor(
            t2, pt, bias_t[oc][:, BI_C2:BI_C2 + 1], aqT[oc],
            op0=ALU.add, op1=ALU.add)                       # aq + m
        nc.vector.tensor_mul(t2, t2, mq_bc)
        outT.append(t2)

    # ---------------- phase 7: final transpose + store ----------------
    otok = transpose_grid("otok", outT, NS, f32, "otok", 2, src_outer=False)
    for sc in range(NS):
        nc.sync.dma_start(out=dram["out"].ap()[sc * P:(sc + 1) * P, :],
                          in_=otok[sc])


def _host_prep(inputs):
    f32 = np.float32
    g1 = np.asarray(inputs["ln1_g"], f32)
    b1 = np.asarray(inputs["ln1_b"], f32)
    g2 = np.asarray(inputs["ln2_g"], f32)
    b2 = np.asarray(inputs["ln2_b"], f32)

    Wq = np.asarray(inputs["Wq"], f32)
    Wk = np.asarray(inputs["Wk"], f32)
    Wv = np.asarray(inputs["Wv"], f32)
    Wo = np.asarray(inputs["Wo"], f32)
    bo = np.asarray(inputs["bo"], f32)
    fcq_w = np.asarray(inputs["fcq_w"], f32)
    fcq_b = np.asarray(inputs["fcq_b"], f32)
    fck_w = np.asarray(inputs["fck_w"], f32)
    fck_b = np.asarray(inputs["fck_b"], f32)
    fcv_w = np.asarray(inputs["fcv_w"], f32)
    fcv_b = np.asarray(inputs["fcv_b"], f32)
    cproj_w = np.asarray(inputs["cproj_w"], f32)
    cproj_b = np.asarray(inputs["cproj_b"], f32)
    cfc_w = np.asarray(inputs["cfc_w"], f32)
    cfc_b = np.asarray(inputs["cfc_b"], f32)
    cproj2_w = np.asarray(inputs["cproj2_w"], f32)
    cproj2_b = np.asarray(inputs["cproj2_b"], f32)
    a_w = [np.asarray(inputs[f"a{j}_w"], f32) for j in (1, 2, 3)]
    a_b = [np.asarray(inputs[f"a{j}_b"], f32) for j in (1, 2, 3)]

    shared = {
        "wq": (g1[:, None] * Wq).astype(BF16),
        "wk": (g1[:, None] * Wk).astype(BF16),
        "wv": (g1[:, None] * Wv).astype(BF16),
        "wo": Wo.astype(BF16),
        "fcq": (g1[:, None] * fcq_w).astype(BF16),
        "fck": (g1[:, None] * fck_w).astype(BF16),
        "fcv": (g1[:, None] * fcv_w).astype(BF16),
        "cpj": cproj_w.astype(BF16),
        # p-major pretiled: [oc, p, a*128+c] with element = W[a*128+p, oc*128+c]
        "gw": np.stack([
            w.reshape(2 * NE, P, NE, P).transpose(2, 1, 0, 3).reshape(
                NE, P, 2 * E)
            for w in a_w]).astype(BF16),
        "cfc": (g2[:, None] * cfc_w).reshape(NE, P, NF, P).transpose(
            2, 1, 0, 3).reshape(NF, P, E).astype(BF16),
        "cp2": (0.5 * cproj2_w).reshape(NF, P, NE, P).transpose(
            2, 1, 0, 3).reshape(NE, P, F).astype(BF16),
    }

    d_q = b1 @ Wq
    d_k = b1 @ Wk
    d_v = b1 @ Wv
    d_cq = b1 @ fcq_w + fcq_b
    d_ck = b1 @ fck_w + fck_b
    d_cv = b1 @ fcv_w + fcv_b
    d_cfc = b2 @ cfc_w + cfc_b

    shared["biasE"] = np.stack(
        [d_q, d_k, bo, d_cq, d_ck, cproj_b, a_b[0], a_b[1], a_b[2],
         cproj2_b], axis=1).astype(f32)
    shared["biasF"] = d_cfc.reshape(NF, P).T.copy().astype(f32)
    shared["dbc"] = np.stack([d_k, d_v, d_cv]).astype(BF16)

    x = np.asarray(inputs["x"], f32)
    enc = np.asarray(inputs["encoder_output"], f32)
    mq = np.asarray(inputs["mask_queries"], f32)
    msa = np.asarray(inputs["mask_self_attention"])
    men = np.asarray(inputs["mask_encoder"])

    in_maps = []
    for b in range(B):
        m8T = np.where(msa[b, 0].T, np.float32(-8e9), np.float32(0.0))
        menc = np.where(men[b, 0, 0], np.float32(-1e4),
                        np.float32(0.0)).reshape(S, 1)
        mqrow = mq[b, :, 0]
        mq2 = np.stack([mqrow / np.sqrt(np.float32(3.0)), mqrow]).astype(f32)
        im = dict(shared)
        im["x"] = np.ascontiguousarray(x[b])
        im["enc"] = np.ascontiguousarray(enc[b])
        im["m8T"] = np.ascontiguousarray(m8T.astype(BF16))
        im["menc"] = np.ascontiguousarray(menc.astype(f32))
        im["mq2"] = np.ascontiguousarray(mq2)
        in_maps.append(im)
    return in_maps


def kernel(**inputs):
    global LAST_RESULTS, LAST_NC
    from concourse import bass_utils

    tau_f = float(np.asarray(inputs["tau"]))
    msa = np.asarray(inputs["mask_self_attention"])
    causal = bool(
        (msa == ~np.tril(np.ones((S, S), bool))[None, None]).all())
    in_maps = _host_prep(inputs)
    nc = _build(tau_f, causal)
    LAST_NC = nc

    trace = bool(int(os.environ.get("KERNEL_TRACE", "0")))
    results = bass_utils.run_bass_kernel_spmd(
        nc, in_maps, core_ids=list(range(B)), trace=trace)
    LAST_RESULTS = results

    out = np.stack([np.asarray(results.results[b]["out"], np.float32)
                    for b in range(B)])
    present = np.stack([np.asarray(results.results[b]["present"], np.float32)
                        for b in range(B)], axis=1)
    return out, present
